# revision 1
# baseline (speedup 1.0000x reference)
"""Trainium2 Bass kernel for nn_CBNNConv2d (binary 3x3 conv, 256ch, 56x56).

Math: the STE forward collapses to  y = conv2d(sign(x), bw)  where
bw = codebook[encoded_vector] reshaped to (O, I, 3, 3), entries +/-1.
The latent `weight` input cancels out of the forward value, and
(sign(x) - clip(x)) + clip(x) rounds back to exactly sign(x) in fp32 —
so the forward is an exact integer convolution of +/-1 operands.
+/-1 is exactly representable in fp8e4, and all partial sums are small
integers, so fp32 PSUM accumulation is exact (measured rel err ~5e-10
vs the fp32 reference; the residual comes from the reference's own
rounding of wb, not from this kernel).

Sharding: data-parallel over batch: 32 images -> 8 cores x 4 images.
The tiny codebook decode runs on host; decoded +/-1 weights are cast to
fp8e4 and replicated to every core (0.3 MB).

Per core (default fp8 DoubleRow variant, cost-model 76.6 us/shot,
DMA-roofline-bound: 25.9 MB HBM traffic ~= 72 us at 358 GB/s):
  - stage ALL 4 images first: DMA x fp32 (1.6 MB per channel-block),
    ScalarE Sign -> fp8 into a zero-padded channel-pair-interleaved
    layout xp[k, f, i] = sign(x)[i*128+k, f] (row pitch 58, borders
    zeroed once, only ~570 border elements re-zeroed per buffer);
    4 pad buffers = no WAR stalls between images
  - conv as matmuls: per output-row chunk (8 rows, N=8*58=464), 9
    DoubleRow matmuls (one per 3x3 tap, K=256 contraction via fp8
    pairs: 2 weights/PE cell, 2 MACs/cycle) accumulate into one PSUM
    bank; rhs slices are contiguous because the output keeps the padded
    row pitch, so each tap is just a shifted flat slice
  - DVE copies PSUM -> SBUF (dropping the 2 junk columns per row);
    output DMAs ride the ACT HWDGE ring so they never head-of-line
    block input DMAs on the SP ring
"""

import os
import time

import numpy as np
import ml_dtypes

O_CH, I_CH, KS = 256, 256, 3
B, H, W = 32, 56, 56
N_CORES = 8
BPC = B // N_CORES  # images per core
PW = H + 2  # padded row pitch = 58
PAD_ROWS = 59  # 58 rows touched + 1 extra row for the +2 tap overrun
PADF = PAD_ROWS * PW  # flat padded length per channel
CHUNK_ROWS = 8
N_CHUNKS = H // CHUNK_ROWS  # 7
NFREE = CHUNK_ROWS * PW  # 464 (<= 512 fp32 per PSUM bank)

_BUILT = None
LAST_RESULT = None


def _build():
    import concourse.tile as tile
    from concourse import bacc, mybir

    f32 = mybir.dt.float32
    bf16 = mybir.dt.bfloat16

    nc = bacc.Bacc(
        "TRN2",
        target_bir_lowering=False,
        debug=False,
        num_devices=N_CORES,
    )
    x_d = nc.dram_tensor("x", [BPC, 2, 128, H, W], f32, kind="ExternalInput").ap()
    w_d = nc.dram_tensor(
        "w", [2, 128, KS, KS, 2, 128], bf16, kind="ExternalInput"
    ).ap()
    y_d = nc.dram_tensor("y", [BPC, 2, 128, H, W], f32, kind="ExternalOutput").ap()

    with tile.TileContext(nc) as tc:
        with (
            tc.tile_pool(name="wpool", bufs=1) as wpool,
            tc.tile_pool(name="xf", bufs=3) as xfp,
            tc.tile_pool(name="pads", bufs=1) as padp,
            tc.tile_pool(name="outp", bufs=3) as outp,
            tc.tile_pool(name="ps", bufs=4, space="PSUM") as psp,
        ):
            w_t = wpool.tile([128, 2, KS, KS, 2, 128], bf16)
            for ib in range(2):
                nc.sync.dma_start(out=w_t[:, ib], in_=w_d[ib])

            # persistent zero-padded sign(x) buffers: [i_blk][phase]
            pads = [
                [
                    padp.tile(
                        [128, PADF], bf16, name=f"pad{ib}{ph}", tag=f"pad{ib}{ph}"
                    )
                    for ph in range(2)
                ]
                for ib in range(2)
            ]
            for ib in range(2):
                for ph in range(2):
                    nc.vector.memset(pads[ib][ph][:], 0.0)

            for img in range(BPC):
                ph = img % 2
                for ib in range(2):
                    xf = xfp.tile([128, H, W], f32)
                    nc.sync.dma_start(out=xf[:], in_=x_d[img, ib])
                    interior = pads[ib][ph].rearrange("p (a b) -> p a b", b=PW)[
                        :, 1 : H + 1, 1 : W + 1
                    ]
                    nc.scalar.sign(interior, xf[:])
                for ob in range(2):
                    o_sb = outp.tile([128, H, W], f32)
                    for c in range(N_CHUNKS):
                        ps = psp.tile([128, NFREE], f32)
                        k = 0
                        for ib in range(2):
                            for kh in range(KS):
                                for kw in range(KS):
                                    off = c * NFREE + kh * PW + kw
                                    nc.tensor.matmul(
                                        ps[:],
                                        lhsT=w_t[:, ib, kh, kw, ob, :],
                                        rhs=pads[ib][ph][:, off : off + NFREE],
                                        start=(k == 0),
                                        stop=(k == 17),
                                    )
                                    k += 1
                        psv = ps.rearrange("p (r w) -> p r w", w=PW)
                        nc.vector.tensor_copy(
                            o_sb[:, c * CHUNK_ROWS : (c + 1) * CHUNK_ROWS, :],
                            psv[:, :, 0:W],
                        )
                    nc.sync.dma_start(out=y_d[img, ob], in_=o_sb[:])
    nc.compile()
    return nc


def _build_fp8(
    repeat=1,
    in_split=1,
    out_every=4,
    psum_bufs=8,
    xf_bufs=6,
    out_bufs=4,
    pad_bufs=4,
    w_first=False,
):
    """fp8e4 DoubleRow variant: channels 0-127 pair with 128-255 on the same
    PE row (2 fp8 weights/cell, 2 MACs/cycle) -> K=256 contraction per matmul,
    9 matmuls per output chunk instead of 18. +/-1 is exact in fp8e4.

    in_split: split each image's input DMA+sign into row-slabs so the PE can
    start on early chunks before the whole image is staged.
    out_every: DMA the output every `out_every` chunks to shrink the drain tail.
    """
    import concourse.tile as tile
    from concourse import bacc, mybir

    f32 = mybir.dt.float32
    fp8 = mybir.dt.float8e4

    nc = bacc.Bacc(
        "TRN2",
        target_bir_lowering=False,
        debug=False,
        num_devices=N_CORES,
    )
    x_d = nc.dram_tensor("x", [BPC, 2, 128, H, W], f32, kind="ExternalInput").ap()
    w_d = nc.dram_tensor(
        "w", [128, KS, KS, 2, 2, 128], fp8, kind="ExternalInput"
    ).ap()
    y_d = nc.dram_tensor("y", [BPC, 2, 128, H, W], f32, kind="ExternalOutput").ap()

    fused_in = in_split == 0  # one 3.2MB DMA per image (both channel blocks)
    if not fused_in:
        assert H % in_split == 0
        slab = H // in_split
    first_split = 4  # stage image 0 in fine slabs so the PE starts early

    with tile.TileContext(nc) as tc:
        with (
            tc.tile_pool(name="wpool", bufs=1) as wpool,
            tc.tile_pool(name="xf", bufs=xf_bufs) as xfp,
            tc.tile_pool(name="pads", bufs=1) as padp,
            tc.tile_pool(name="outp", bufs=out_bufs) as outp,
            tc.tile_pool(name="ps", bufs=psum_bufs, space="PSUM") as psp,
        ):
            w_t = wpool.tile([128, KS, KS, 2, 2, 128], fp8)
            if w_first:
                nc.sync.dma_start(out=w_t[:], in_=w_d[:])

            # PE warmup: keep the tensor engine busy through the initial DMA
            # wait so the HAM clock gate is at 8/8 when real matmuls start.
            # Writes only a scratch PSUM bank that is never read.
            warm_src = wpool.tile([128, 64], fp8, name="warm_src")
            nc.vector.memset(warm_src[:], 1.0)
            warm_ps = psp.tile([128, NFREE], f32, name="warm_ps", tag="ps")
            for _ in range(100):
                nc.tensor.matmul(
                    warm_ps[0:64, 0:64],
                    lhsT=warm_src[:, 0:64],
                    rhs=warm_src[:, 0:64],
                    start=True,
                    stop=True,
                )

            # padded sign(x) in channel-pair-interleaved layout:
            # xp[k, f, i] = sign(x)[i*128 + k, spatial f]  (f in padded coords)
            pads = [
                padp.tile([128, PADF, 2], fp8, name=f"padp{ph}", tag=f"padp{ph}")
                for ph in range(pad_bufs)
            ]
            for ph in range(pad_bufs):
                xp = pads[ph]
                # zero only the padding border (the interior is rewritten by
                # Sign every image): head = row 0 + (row1,col0); the seam
                # [row r col 57 .. row r+1 col 0] for r=1..55 (4 fp8 els each);
                # tail = (row56,col57) onward through rows 57-58.
                nc.vector.memset(xp[:, 0 : PW + 1, :], 0.0)
                seam = xp.rearrange("p (a b) i -> p a b i", b=PW)
                nc.vector.memset(seam[:, 1:56, W + 1 : W + 2, :], 0.0)
                nc.vector.memset(seam[:, 1:57, 0:1, :], 0.0)
                nc.vector.memset(xp[:, 56 * PW + W + 1 :, :], 0.0)

            if not w_first:
                # ACT HWDGE ring: keeps the SP ring free for the first x DMA
                nc.scalar.dma_start(out=w_t[:], in_=w_d[:])

            for rep in range(repeat):
                for img in range(BPC):
                    ph = img % pad_bufs
                    xp = pads[ph]
                    xp4 = xp.rearrange("p (a b) i -> p a b i", b=PW)
                    if fused_in:
                        xf = xfp.tile(
                            [128, 2, H, W], f32, name=f"xff{img}", tag="xf"
                        )
                        nc.sync.dma_start(
                            out=xf[:],
                            in_=x_d[img].rearrange("i p a b -> p i a b"),
                        )
                        for ib in range(2):
                            nc.scalar.sign(
                                xp4[:, 1 : H + 1, 1 : W + 1, ib], xf[:, ib]
                            )
                    else:
                        nsplit = first_split if (img == 0 and rep == 0) else in_split
                        sl = H // nsplit
                        bounds = [s * sl for s in range(nsplit)] + [H]
                        for s, (r0, r1) in enumerate(
                            zip(bounds[:-1], bounds[1:])
                        ):
                            for ib in range(2):
                                xf = xfp.tile(
                                    [128, r1 - r0, W], f32,
                                    name=f"xf{img}{s}{ib}", tag="xf",
                                )
                                # very first slab: put ib=1 on the ACT HWDGE
                                # ring so both halves land concurrently
                                eng = (
                                    nc.scalar
                                    if (img == 0 and rep == 0 and s == 0 and ib == 1)
                                    else nc.sync
                                )
                                eng.dma_start(
                                    out=xf[:], in_=x_d[img, ib, :, r0:r1]
                                )
                                nc.scalar.sign(
                                    xp4[:, 1 + r0 : 1 + r1, 1 : W + 1, ib],
                                    xf[:],
                                )
                for img in range(BPC):
                    ph = img % pad_bufs
                    xp = pads[ph]
                    _emit_image_compute(
                        nc, mybir, psp, outp, w_t, xp, y_d, img, out_every, f32
                    )
    nc.compile()
    return nc


def _emit_image_compute(nc, mybir, psp, outp, w_t, xp, y_d, img, out_every, f32):
    for ob in range(2):
        o_sb = outp.tile([128, H, W], f32, name=f"osb{img}{ob}", tag="osb")
        done = 0
        for c in range(N_CHUNKS):
            ps = psp.tile([128, NFREE], f32, name=f"ps{img}{ob}{c}", tag="ps")
            k = 0
            for kh in range(KS):
                for kw in range(KS):
                    off = c * NFREE + kh * PW + kw
                    rhs = xp[:, off : off + NFREE, :].rearrange("p n i -> p i n")
                    nc.tensor.matmul(
                        ps[:],
                        lhsT=w_t[:, kh, kw, ob],
                        rhs=rhs,
                        start=(k == 0),
                        stop=(k == 8),
                        perf_mode=mybir.MatmulPerfMode.DoubleRow,
                    )
                    k += 1
            psv = ps.rearrange("p (r w) -> p r w", w=PW)
            nc.vector.tensor_copy(
                o_sb[:, c * CHUNK_ROWS : (c + 1) * CHUNK_ROWS, :],
                psv[:, :, 0:W],
            )
            last = img == BPC - 1 and ob == 1
            flush = (
                (c + 1) in (4, 6, 7)
                if last  # taper the final drain: 32/16/8-row DMAs
                else ((c + 1) % out_every == 0 or c == N_CHUNKS - 1)
            )
            if flush:
                h0, h1 = done * CHUNK_ROWS, (c + 1) * CHUNK_ROWS
                nc.scalar.dma_start(
                    out=y_d[img, ob, :, h0:h1],
                    in_=o_sb[:, done * CHUNK_ROWS : h1, :],
                )
                done = c + 1


def _decode_weights(codebook, encoded_vector):
    bw = codebook[encoded_vector].reshape(-1)[: O_CH * I_CH * KS * KS]
    bw = bw.reshape(O_CH, I_CH, KS, KS)
    # [i_blk, k(part), kh, kw, o_blk, m] : lhsT layout (contraction on partitions)
    wt = bw.transpose(1, 2, 3, 0).reshape(2, 128, KS, KS, 2, 128)
    return np.ascontiguousarray(wt).astype(ml_dtypes.bfloat16)


def _decode_weights_fp8(codebook, encoded_vector):
    bw = codebook[encoded_vector].reshape(-1)[: O_CH * I_CH * KS * KS]
    bw = bw.reshape(O_CH, I_CH, KS, KS)
    wt = bw.transpose(1, 2, 3, 0).reshape(2, 128, KS, KS, 2, 128)
    # -> [k(part), kh, kw, o_blk, i_blk(pair), m]
    w2 = wt.transpose(1, 2, 3, 4, 0, 5)
    return np.ascontiguousarray(w2).astype(ml_dtypes.float8_e4m3)


def kernel(x, weight, codebook, encoded_vector):
    global _BUILT, LAST_RESULT
    from concourse import bass_utils

    x = np.ascontiguousarray(np.asarray(x, dtype=np.float32))
    codebook = np.asarray(codebook, dtype=np.float32)
    encoded_vector = np.asarray(encoded_vector)

    use_bf16 = os.environ.get("KERNEL_VARIANT", "fp8") == "bf16"
    if _BUILT is None:
        _BUILT = _build() if use_bf16 else _build_fp8()
    nc = _BUILT

    if use_bf16:
        wt = _decode_weights(codebook, encoded_vector)
    else:
        wt = _decode_weights_fp8(codebook, encoded_vector)
    x8 = x.reshape(N_CORES, BPC, 2, 128, H, W)
    in_maps = [{"x": x8[i], "w": wt} for i in range(N_CORES)]

    trace = bool(int(os.environ.get("KERNEL_TRACE", "0")))

    def _run(tr):
        return bass_utils.run_bass_kernel_spmd(
            nc, in_maps, core_ids=list(range(N_CORES)), trace=tr
        )

    res = None
    for attempt in range(3):
        try:
            res = _run(trace)
            break
        except ModuleNotFoundError:
            # axon client without the NTFF profile hook: disable tracing
            os.environ["BASS_NEVER_TRACE"] = "1"
            trace = False
        except Exception:
            # transient device errors (NRT_EXEC_UNIT_UNRECOVERABLE) recover
            # on retry
            if attempt == 2:
                raise
            time.sleep(5)
    if res is None:
        res = _run(trace)
    LAST_RESULT = res
    y = np.stack([res.results[i]["y"] for i in range(N_CORES)], axis=0)
    return np.ascontiguousarray(y.reshape(B, O_CH, H, W))



# revision 39
# speedup vs baseline: 1.3035x; 1.3035x over previous
"""Trainium2 Bass kernel for nn_CBNNConv2d (binary 3x3 conv, 256ch, 56x56).

Math: the STE forward collapses to  y = conv2d(sign(x), bw)  where
bw = codebook[encoded_vector] reshaped to (O, I, 3, 3), entries +/-1.
The latent `weight` input cancels out of the forward value, and
(sign(x) - clip(x)) + clip(x) rounds back to exactly sign(x) in fp32 —
so the forward is an exact integer convolution of +/-1 operands.
+/-1 is exactly representable in fp8e4, and all partial sums are small
integers (|y| <= 2304), so fp32 PSUM accumulation is exact and an fp16
output loses essentially nothing (integers <= 2048 are exact in fp16;
measured rel err ~5e-10 vs the fp32 reference, identical to an fp32
output — the residual comes from the reference's own rounding of wb).

Sharding: data-parallel over batch: 32 images -> 8 cores x 4 images.
The tiny codebook decode runs on host; decoded +/-1 weights are cast to
fp8e4 and replicated to every core (0.58 MB).

Per core the shot is DMA-bound: 12.85 MB x in (fp32) + 6.42 MB y out
(fp16) + 0.58 MB weights = 19.86 MB through the ~360 B/ns HBM<->SBUF
path =~ 55.2 us, and the schedule keeps that stream gap-free
(cost-model 58.6 us/shot vs 76.4 us for the fp32-out baseline):
  - x streams in row-slabs (10-row slabs at the head so compute starts
    ~4 us in, 14-row after — a slab's transfer must outlast the ~650 ns
    HWDGE descriptor generation or the DMA stream gaps); ScalarE Sign
    -> fp8 into a zero-padded channel-pair-interleaved layout
    xp[k, f, i] = sign(x)[i*128+k, f] (row pitch 58, borders zeroed
    once); 4 pad buffers = no WAR stalls between images
  - the weight DMA is split per output-channel block and rides the SP
    ring right behind the first slab pair: the first matmul only needs
    the ob=0 half
  - conv as matmuls: per output-row chunk (8 rows x 56 cols, N=448), 9
    DoubleRow matmuls (one per 3x3 tap, K=256 contraction via fp8
    pairs: 2 weights/PE cell) accumulate into one PSUM bank; the rhs is
    a 3-free-dim AP (pair, row, col) over the padded buffer, so the 2
    pad columns per row are never computed; the two output-channel
    blocks of a chunk are interleaved so the PE's input-row consumption
    never outruns the arriving slabs
  - DVE copies PSUM -> SBUF fp16; ALL output DMAs ride the SP ring and
    are emitted after every x slab, so the FIFO ring gives the input
    stream strict priority on the (serialized) DMA engines — an output
    flush wedged between x slabs would delay the last image and with it
    the whole tail.  By the time the SP sequencer reaches a flush its
    DVE copy has completed, so nothing head-of-line blocks.  8 output
    buffers keep compute from ever waiting on the late flushes.
"""

import os
import time

import numpy as np
import ml_dtypes

O_CH, I_CH, KS = 256, 256, 3
B, H, W = 32, 56, 56
N_CORES = 8
BPC = B // N_CORES  # images per core
PW = H + 2  # padded row pitch = 58
PAD_ROWS = 59  # 58 rows touched + 1 extra row for the +2 tap overrun
PADF = PAD_ROWS * PW  # flat padded length per channel
CHUNK_ROWS = 8
N_CHUNKS = H // CHUNK_ROWS  # 7
NFREE = CHUNK_ROWS * PW  # 464 (<= 512 fp32 per PSUM bank)
NOUT = CHUNK_ROWS * W  # 448: matmul free dim without the 2 junk cols/row

_BUILT = None
LAST_RESULT = None


def _build():
    import concourse.tile as tile
    from concourse import bacc, mybir

    f32 = mybir.dt.float32
    bf16 = mybir.dt.bfloat16

    nc = bacc.Bacc(
        "TRN2",
        target_bir_lowering=False,
        debug=False,
        num_devices=N_CORES,
    )
    x_d = nc.dram_tensor("x", [BPC, 2, 128, H, W], f32, kind="ExternalInput").ap()
    w_d = nc.dram_tensor(
        "w", [2, 128, KS, KS, 2, 128], bf16, kind="ExternalInput"
    ).ap()
    y_d = nc.dram_tensor("y", [BPC, 2, 128, H, W], f32, kind="ExternalOutput").ap()

    with tile.TileContext(nc) as tc:
        with (
            tc.tile_pool(name="wpool", bufs=1) as wpool,
            tc.tile_pool(name="xf", bufs=3) as xfp,
            tc.tile_pool(name="pads", bufs=1) as padp,
            tc.tile_pool(name="outp", bufs=3) as outp,
            tc.tile_pool(name="ps", bufs=4, space="PSUM") as psp,
        ):
            w_t = wpool.tile([128, 2, KS, KS, 2, 128], bf16)
            for ib in range(2):
                nc.sync.dma_start(out=w_t[:, ib], in_=w_d[ib])

            # persistent zero-padded sign(x) buffers: [i_blk][phase]
            pads = [
                [
                    padp.tile(
                        [128, PADF], bf16, name=f"pad{ib}{ph}", tag=f"pad{ib}{ph}"
                    )
                    for ph in range(2)
                ]
                for ib in range(2)
            ]
            for ib in range(2):
                for ph in range(2):
                    nc.vector.memset(pads[ib][ph][:], 0.0)

            for img in range(BPC):
                ph = img % 2
                for ib in range(2):
                    xf = xfp.tile([128, H, W], f32)
                    nc.sync.dma_start(out=xf[:], in_=x_d[img, ib])
                    interior = pads[ib][ph].rearrange("p (a b) -> p a b", b=PW)[
                        :, 1 : H + 1, 1 : W + 1
                    ]
                    nc.scalar.sign(interior, xf[:])
                for ob in range(2):
                    o_sb = outp.tile([128, H, W], f32)
                    for c in range(N_CHUNKS):
                        ps = psp.tile([128, NFREE], f32)
                        k = 0
                        for ib in range(2):
                            for kh in range(KS):
                                for kw in range(KS):
                                    off = c * NFREE + kh * PW + kw
                                    nc.tensor.matmul(
                                        ps[:],
                                        lhsT=w_t[:, ib, kh, kw, ob, :],
                                        rhs=pads[ib][ph][:, off : off + NFREE],
                                        start=(k == 0),
                                        stop=(k == 17),
                                    )
                                    k += 1
                        psv = ps.rearrange("p (r w) -> p r w", w=PW)
                        nc.vector.tensor_copy(
                            o_sb[:, c * CHUNK_ROWS : (c + 1) * CHUNK_ROWS, :],
                            psv[:, :, 0:W],
                        )
                    nc.sync.dma_start(out=y_d[img, ob], in_=o_sb[:])
    nc.compile()
    return nc


def _build_fp8(
    repeat=1,
    in_split=4,
    out_every=4,
    psum_bufs=8,
    xf_bufs=8,
    out_bufs=8,
    pad_bufs=4,
    warmup=100,
    tail_ring="sync",
    flush_ring="sync",
    ob_interleave_n=4,
):
    """fp8e4 DoubleRow variant: channels 0-127 pair with 128-255 on the same
    PE row (2 fp8 weights/cell, 2 MACs/cycle) -> K=256 contraction per matmul,
    9 matmuls per output chunk instead of 18. +/-1 is exact in fp8e4.

    in_split: split each image's input DMA+sign into row-slabs so the PE can
    start on early chunks before the whole image is staged.
    out_every: DMA the output every `out_every` chunks to shrink the drain tail.
    """
    import concourse.tile as tile
    from concourse import bacc, mybir

    f32 = mybir.dt.float32
    f16 = mybir.dt.float16
    fp8 = mybir.dt.float8e4

    nc = bacc.Bacc(
        "TRN2",
        target_bir_lowering=False,
        debug=False,
        num_devices=N_CORES,
    )
    x_d = nc.dram_tensor("x", [BPC, 2, 128, H, W], f32, kind="ExternalInput").ap()
    # ob-major so each output-channel block's weights are one contiguous DMA:
    # the first matmul only needs the ob=0 half, so ob=1 streams in later.
    w_d = nc.dram_tensor(
        "w", [2, 128, KS, KS, 2, 128], fp8, kind="ExternalInput"
    ).ap()
    # fp16 output: every conv value is an integer in [-2304, 2304]; fp16 is
    # exact up to 2048 and rounds to even multiples of 2 beyond, so the
    # added error is ~0 while the output DMA traffic halves.
    y_d = nc.dram_tensor("y", [BPC, 2, 128, H, W], f16, kind="ExternalOutput").ap()

    fused_in = in_split == 0  # one 3.2MB DMA per image (both channel blocks)
    if not fused_in:
        assert H % in_split == 0

    with tile.TileContext(nc) as tc:
        with (
            tc.tile_pool(name="wpool", bufs=1) as wpool,
            tc.tile_pool(name="xf", bufs=xf_bufs) as xfp,
            tc.tile_pool(name="pads", bufs=1) as padp,
            tc.tile_pool(name="outp", bufs=out_bufs) as outp,
            tc.tile_pool(name="ps", bufs=psum_bufs, space="PSUM") as psp,
        ):
            w_t = wpool.tile([128, 2, KS, KS, 2, 128], fp8)

            # PE warmup: keep the tensor engine busy through the initial DMA
            # wait so the HAM clock gate is at 8/8 when real matmuls start.
            # Writes only a scratch PSUM bank that is never read.
            if warmup:
                warm_src = wpool.tile([128, 64], fp8, name="warm_src")
                nc.vector.memset(warm_src[:], 1.0)
                warm_ps = psp.tile([128, NFREE], f32, name="warm_ps", tag="ps")
                for _ in range(warmup):
                    nc.tensor.matmul(
                        warm_ps[0:64, 0:64],
                        lhsT=warm_src[:, 0:64],
                        rhs=warm_src[:, 0:64],
                        start=True,
                        stop=True,
                    )

            # padded sign(x) in channel-pair-interleaved layout:
            # xp[k, f, i] = sign(x)[i*128 + k, spatial f]  (f in padded coords)
            pads = [
                padp.tile([128, PADF, 2], fp8, name=f"padp{ph}", tag=f"padp{ph}")
                for ph in range(pad_bufs)
            ]
            for ph in range(pad_bufs):
                xp = pads[ph]
                # zero only the padding border (the interior is rewritten by
                # Sign every image): head = row 0 + (row1,col0); the seam
                # [row r col 57 .. row r+1 col 0] for r=1..55 (4 fp8 els each);
                # tail = (row56,col57) onward through rows 57-58.
                nc.vector.memset(xp[:, 0 : PW + 1, :], 0.0)
                seam = xp.rearrange("p (a b) i -> p a b i", b=PW)
                nc.vector.memset(seam[:, 1:56, W + 1 : W + 2, :], 0.0)
                nc.vector.memset(seam[:, 1:57, 0:1, :], 0.0)
                nc.vector.memset(xp[:, 56 * PW + W + 1 :, :], 0.0)

            if fused_in:
                nc.sync.dma_start(out=w_t[:, 0], in_=w_d[0])
                nc.sync.dma_start(out=w_t[:, 1], in_=w_d[1])

            for rep in range(repeat):
                for img in range(BPC):
                    ph = img % pad_bufs
                    xp = pads[ph]
                    xp4 = xp.rearrange("p (a b) i -> p a b i", b=PW)
                    if fused_in:
                        xf = xfp.tile(
                            [128, 2, H, W], f32, name=f"xff{img}", tag="xf"
                        )
                        nc.sync.dma_start(
                            out=xf[:],
                            in_=x_d[img].rearrange("i p a b -> p i a b"),
                        )
                        for ib in range(2):
                            nc.scalar.sign(
                                xp4[:, 1 : H + 1, 1 : W + 1, ib], xf[:, ib]
                            )
                    else:
                        if img == 0 and rep == 0:
                            # 10-row slabs: transfer (797ns) stays above HWDGE
                            # descriptor generation (~657ns) so the DMA stream
                            # has no gaps, and chunk c is ready after slab c+1
                            bounds = [0, 10, 18, 26, 34, 42, 56]
                        else:
                            sl = H // in_split
                            bounds = [s * sl for s in range(in_split)] + [H]
                        for s, (r0, r1) in enumerate(
                            zip(bounds[:-1], bounds[1:])
                        ):
                            for ib in range(2):
                                xf = xfp.tile(
                                    [128, r1 - r0, W], f32,
                                    name=f"xf{img}{s}{ib}", tag="xf",
                                )
                                nc.sync.dma_start(
                                    out=xf[:], in_=x_d[img, ib, :, r0:r1]
                                )
                                nc.scalar.sign(
                                    xp4[:, 1 + r0 : 1 + r1, 1 : W + 1, ib],
                                    xf[:],
                                )
                            if img == 0 and rep == 0 and s == 0:
                                # weights ride the SP ring right behind the
                                # first slab pair: the first matmul needs
                                # only the ob=0 half, ob=1 follows
                                nc.sync.dma_start(out=w_t[:, 0], in_=w_d[0])
                                nc.sync.dma_start(out=w_t[:, 1], in_=w_d[1])

                for img in range(BPC):
                    ph = img % pad_bufs
                    xp = pads[ph]
                    _emit_image_compute(
                        nc, mybir, psp, outp, w_t, xp, y_d, img, out_every,
                        f16, tail_ring, flush_ring,
                        ob_interleave=(img < ob_interleave_n),
                    )
    nc.compile()
    return nc


def _emit_image_compute(
    nc,
    mybir,
    psp,
    outp,
    w_t,
    xp,
    y_d,
    img,
    out_every,
    o_dt,
    tail_ring="sync",
    flush_ring="sync",
    ob_interleave=False,
):
    """Emit matmuls + PSUM drain + output DMA for one image.

    ob_interleave: do chunk c for both output blocks before chunk c+1 —
    halves the PE's input-row consumption rate, so early images that are
    still streaming in from HBM don't starve the PE.
    """
    f32 = mybir.dt.float32
    o_sb = [
        outp.tile([128, H, W], o_dt, name=f"osb{img}{ob}", tag="osb")
        for ob in range(2)
    ]
    done = [0, 0]

    xp4 = xp.rearrange("p (a b) i -> p a b i", b=PW)

    def emit_chunk(ob, c):
        ps = psp.tile([128, NOUT], f32, name=f"ps{img}{ob}{c}", tag="ps")
        k = 0
        for kh in range(KS):
            for kw in range(KS):
                # exact 8x56 output window: rhs is a 3-free-dim AP (pair,
                # row, col) — 448-wide matmuls instead of 464 (junk cols)
                r0 = c * CHUNK_ROWS + kh
                rhs = xp4[:, r0 : r0 + CHUNK_ROWS, kw : kw + W, :].rearrange(
                    "p r w i -> p i r w"
                )
                nc.tensor.matmul(
                    ps[:],
                    lhsT=w_t[:, ob, kh, kw],
                    rhs=rhs,
                    start=(k == 0),
                    stop=(k == 8),
                    perf_mode=mybir.MatmulPerfMode.DoubleRow,
                )
                k += 1
        nc.vector.tensor_copy(
            o_sb[ob][:, c * CHUNK_ROWS : (c + 1) * CHUNK_ROWS, :],
            ps.rearrange("p (r w) -> p r w", w=W),
        )
        last = img == BPC - 1 and ob == 1
        flush = (
            (c + 1) in (4, 6, 7)
            if last  # taper the final drain: 32/16/8-row DMAs
            else ((c + 1) % out_every == 0 or c == N_CHUNKS - 1)
        )
        if flush:
            h0, h1 = done[ob] * CHUNK_ROWS, (c + 1) * CHUNK_ROWS
            # All flushes ride the SP ring, emitted AFTER every x slab: the
            # FIFO ring gives the input stream strict priority on the shared
            # DMA engines (a y flush sneaking in between x slabs delays the
            # last image's input and with it the tail of the whole kernel).
            # By the time the SP sequencer reaches a flush, its DVE copy has
            # long completed, so the semaphore wait does not head-of-line
            # block anything.
            eng = getattr(nc, tail_ring) if last else getattr(nc, flush_ring)
            eng.dma_start(
                out=y_d[img, ob, :, h0:h1],
                in_=o_sb[ob][:, h0:h1, :],
            )
            done[ob] = c + 1

    if ob_interleave:
        for c in range(N_CHUNKS):
            for ob in range(2):
                emit_chunk(ob, c)
    else:
        for ob in range(2):
            for c in range(N_CHUNKS):
                emit_chunk(ob, c)


def _decode_weights(codebook, encoded_vector):
    bw = codebook[encoded_vector].reshape(-1)[: O_CH * I_CH * KS * KS]
    bw = bw.reshape(O_CH, I_CH, KS, KS)
    # [i_blk, k(part), kh, kw, o_blk, m] : lhsT layout (contraction on partitions)
    wt = bw.transpose(1, 2, 3, 0).reshape(2, 128, KS, KS, 2, 128)
    return np.ascontiguousarray(wt).astype(ml_dtypes.bfloat16)


def _decode_weights_fp8(codebook, encoded_vector):
    bw = codebook[encoded_vector].reshape(-1)[: O_CH * I_CH * KS * KS]
    bw = bw.reshape(O_CH, I_CH, KS, KS)
    wt = bw.transpose(1, 2, 3, 0).reshape(2, 128, KS, KS, 2, 128)
    # -> [o_blk, k(part), kh, kw, i_blk(pair), m]
    w2 = wt.transpose(4, 1, 2, 3, 0, 5)
    return np.ascontiguousarray(w2).astype(ml_dtypes.float8_e4m3)


def kernel(x, weight, codebook, encoded_vector):
    global _BUILT, LAST_RESULT
    from concourse import bass_utils

    x = np.ascontiguousarray(np.asarray(x, dtype=np.float32))
    codebook = np.asarray(codebook, dtype=np.float32)
    encoded_vector = np.asarray(encoded_vector)

    use_bf16 = os.environ.get("KERNEL_VARIANT", "fp8") == "bf16"
    if _BUILT is None:
        _BUILT = _build() if use_bf16 else _build_fp8()
    nc = _BUILT

    if use_bf16:
        wt = _decode_weights(codebook, encoded_vector)
    else:
        wt = _decode_weights_fp8(codebook, encoded_vector)
    x8 = x.reshape(N_CORES, BPC, 2, 128, H, W)
    in_maps = [{"x": x8[i], "w": wt} for i in range(N_CORES)]

    trace = bool(int(os.environ.get("KERNEL_TRACE", "0")))

    def _run(tr):
        return bass_utils.run_bass_kernel_spmd(
            nc, in_maps, core_ids=list(range(N_CORES)), trace=tr
        )

    res = None
    for attempt in range(3):
        try:
            res = _run(trace)
            break
        except ModuleNotFoundError:
            # axon client without the NTFF profile hook: disable tracing
            os.environ["BASS_NEVER_TRACE"] = "1"
            trace = False
        except Exception:
            # transient device errors (NRT_EXEC_UNIT_UNRECOVERABLE) recover
            # on retry
            if attempt == 2:
                raise
            time.sleep(5)
    if res is None:
        res = _run(trace)
    LAST_RESULT = res
    y = np.stack([res.results[i]["y"] for i in range(N_CORES)], axis=0)
    return np.ascontiguousarray(y.reshape(B, O_CH, H, W).astype(np.float32))



# revision 56
# speedup vs baseline: 1.3547x; 1.0393x over previous
"""Trainium2 Bass kernel for nn_CBNNConv2d (binary 3x3 conv, 256ch, 56x56).

Math: the STE forward collapses to  y = conv2d(sign(x), bw)  where
bw = codebook[encoded_vector] reshaped to (O, I, 3, 3), entries +/-1.
The latent `weight` input cancels out of the forward value, and
(sign(x) - clip(x)) + clip(x) rounds back to exactly sign(x) in fp32 —
so the forward is an exact integer convolution of +/-1 operands.
+/-1 is exactly representable in fp8e4, and all partial sums are small
integers (|y| <= 2304), so fp32 PSUM accumulation is exact and an fp16
output loses essentially nothing (integers <= 2048 are exact in fp16;
measured rel err ~5e-10 vs the fp32 reference, identical to an fp32
output — the residual comes from the reference's own rounding of wb).

Sharding: data-parallel over batch: 32 images -> 8 cores x 4 images.
The tiny codebook decode runs on host; decoded +/-1 weights are cast to
fp8e4 and replicated to every core (0.58 MB).

Per core the shot is PE-bound: 504 DoubleRow matmuls x 448 cols x 0.5
cyc/col at 2.4 GHz = 47 us of tensor-engine time, run gap-free between
a ~5.4 us data-latency head and a ~4 us drain tail (cost-model 56.5
us/shot vs 76.4 us for the original fp32-I/O baseline):
  - x streams in row-slabs as CASTING SWDGE DMAs on the Pool ring: the
    software DGE converts fp32 -> bf16 in flight (sign bits preserved
    exactly), halving both the modeled transfer time and SBUF traffic;
    both channel blocks ride one DMA per slab so descriptor generation
    (~1.04 us serial on the Pool engine) stays under the transfer time.
    DMA totals: 6.42 MB x in (bf16) + 6.42 MB y out (fp16) + 0.58 MB
    weights = 13.4 MB =~ 37 us at the 360 B/ns serialized DMA device —
    comfortably under the PE time.
  - one fused ScalarE Sign per slab (both channel blocks in one op)
    writes fp8 +/-1 into a zero-padded channel-pair-interleaved layout
    xp[k, f, i] = sign(x)[i*128+k, f] (row pitch 58, borders zeroed
    once); 4 pad buffers = no WAR stalls between images.  Every
    DMA->compute edge pays a fixed 900 ns completion-semaphore delay,
    so the head uses a small 10-row first slab.
  - the weight DMA rides the otherwise-idle SP HWDGE ring, ob=0 half
    split per-kh so the first matmul's taps land first
  - conv as matmuls: per output-row chunk (8 rows x 56 cols, N=448), 9
    DoubleRow matmuls (one per 3x3 tap, K=256 contraction via fp8
    pairs: 2 weights/PE cell) accumulate into one PSUM bank; the rhs is
    a 3-free-dim AP (pair, row, col) over the padded buffer, so the 2
    pad columns per row are never computed; the two output-channel
    blocks of a chunk are interleaved so the PE's input-row consumption
    never outruns the arriving slabs
  - DVE copies PSUM -> SBUF fp16; output DMAs ride the SP ring (their
    semaphore waits are satisfied by dispatch time, so nothing
    head-of-line blocks), with the final flushes tapered 32/16/8 rows
    to shrink the drain tail.  8 output buffers keep compute from ever
    waiting on flushes.
"""

import os
import time

import numpy as np
import ml_dtypes

O_CH, I_CH, KS = 256, 256, 3
B, H, W = 32, 56, 56
N_CORES = 8
BPC = B // N_CORES  # images per core
PW = H + 2  # padded row pitch = 58
PAD_ROWS = 59  # 58 rows touched + 1 extra row for the +2 tap overrun
PADF = PAD_ROWS * PW  # flat padded length per channel
CHUNK_ROWS = 8
N_CHUNKS = H // CHUNK_ROWS  # 7
NFREE = CHUNK_ROWS * PW  # 464 (<= 512 fp32 per PSUM bank)
NOUT = CHUNK_ROWS * W  # 448: matmul free dim without the 2 junk cols/row

_BUILT = None
LAST_RESULT = None


def _build():
    import concourse.tile as tile
    from concourse import bacc, mybir

    f32 = mybir.dt.float32
    bf16 = mybir.dt.bfloat16

    nc = bacc.Bacc(
        "TRN2",
        target_bir_lowering=False,
        debug=False,
        num_devices=N_CORES,
    )
    x_d = nc.dram_tensor("x", [BPC, 2, 128, H, W], f32, kind="ExternalInput").ap()
    w_d = nc.dram_tensor(
        "w", [2, 128, KS, KS, 2, 128], bf16, kind="ExternalInput"
    ).ap()
    y_d = nc.dram_tensor("y", [BPC, 2, 128, H, W], f32, kind="ExternalOutput").ap()

    with tile.TileContext(nc) as tc:
        with (
            tc.tile_pool(name="wpool", bufs=1) as wpool,
            tc.tile_pool(name="xf", bufs=3) as xfp,
            tc.tile_pool(name="pads", bufs=1) as padp,
            tc.tile_pool(name="outp", bufs=3) as outp,
            tc.tile_pool(name="ps", bufs=4, space="PSUM") as psp,
        ):
            w_t = wpool.tile([128, 2, KS, KS, 2, 128], bf16)
            for ib in range(2):
                nc.sync.dma_start(out=w_t[:, ib], in_=w_d[ib])

            # persistent zero-padded sign(x) buffers: [i_blk][phase]
            pads = [
                [
                    padp.tile(
                        [128, PADF], bf16, name=f"pad{ib}{ph}", tag=f"pad{ib}{ph}"
                    )
                    for ph in range(2)
                ]
                for ib in range(2)
            ]
            for ib in range(2):
                for ph in range(2):
                    nc.vector.memset(pads[ib][ph][:], 0.0)

            for img in range(BPC):
                ph = img % 2
                for ib in range(2):
                    xf = xfp.tile([128, H, W], f32)
                    nc.sync.dma_start(out=xf[:], in_=x_d[img, ib])
                    interior = pads[ib][ph].rearrange("p (a b) -> p a b", b=PW)[
                        :, 1 : H + 1, 1 : W + 1
                    ]
                    nc.scalar.sign(interior, xf[:])
                for ob in range(2):
                    o_sb = outp.tile([128, H, W], f32)
                    for c in range(N_CHUNKS):
                        ps = psp.tile([128, NFREE], f32)
                        k = 0
                        for ib in range(2):
                            for kh in range(KS):
                                for kw in range(KS):
                                    off = c * NFREE + kh * PW + kw
                                    nc.tensor.matmul(
                                        ps[:],
                                        lhsT=w_t[:, ib, kh, kw, ob, :],
                                        rhs=pads[ib][ph][:, off : off + NFREE],
                                        start=(k == 0),
                                        stop=(k == 17),
                                    )
                                    k += 1
                        psv = ps.rearrange("p (r w) -> p r w", w=PW)
                        nc.vector.tensor_copy(
                            o_sb[:, c * CHUNK_ROWS : (c + 1) * CHUNK_ROWS, :],
                            psv[:, :, 0:W],
                        )
                    nc.sync.dma_start(out=y_d[img, ob], in_=o_sb[:])
    nc.compile()
    return nc


def _build_fp8(
    repeat=1,
    in_split=4,
    out_every=4,
    psum_bufs=8,
    xf_bufs=8,
    out_bufs=8,
    pad_bufs=4,
    warmup=100,
    tail_ring="sync",
    flush_ring="sync",
    ob_interleave_n=3,
    first_bounds=(0, 10, 18, 26, 34, 42, 56),
    split_c0=False,
):
    """fp8e4 DoubleRow variant: channels 0-127 pair with 128-255 on the same
    PE row (2 fp8 weights/cell, 2 MACs/cycle) -> K=256 contraction per matmul,
    9 matmuls per output chunk instead of 18. +/-1 is exact in fp8e4.

    in_split: split each image's input DMA+sign into row-slabs so the PE can
    start on early chunks before the whole image is staged.
    out_every: DMA the output every `out_every` chunks to shrink the drain tail.
    """
    import concourse.tile as tile
    from concourse import bacc, mybir

    f32 = mybir.dt.float32
    f16 = mybir.dt.float16
    bf16 = mybir.dt.bfloat16
    fp8 = mybir.dt.float8e4

    nc = bacc.Bacc(
        "TRN2",
        target_bir_lowering=False,
        debug=False,
        num_devices=N_CORES,
    )
    x_d = nc.dram_tensor("x", [BPC, 2, 128, H, W], f32, kind="ExternalInput").ap()
    # ob-major so each output-channel block's weights are one contiguous DMA:
    # the first matmul only needs the ob=0 half, so ob=1 streams in later.
    w_d = nc.dram_tensor(
        "w", [2, 128, KS, KS, 2, 128], fp8, kind="ExternalInput"
    ).ap()
    # fp16 output: every conv value is an integer in [-2304, 2304]; fp16 is
    # exact up to 2048 and rounds to even multiples of 2 beyond, so the
    # added error is ~0 while the output DMA traffic halves.
    y_d = nc.dram_tensor("y", [BPC, 2, 128, H, W], f16, kind="ExternalOutput").ap()

    fused_in = in_split == 0  # one 3.2MB DMA per image (both channel blocks)
    if not fused_in:
        assert H % in_split == 0

    with tile.TileContext(nc) as tc:
        with (
            tc.tile_pool(name="wpool", bufs=1) as wpool,
            tc.tile_pool(name="xf", bufs=xf_bufs) as xfp,
            tc.tile_pool(name="pads", bufs=1) as padp,
            tc.tile_pool(name="outp", bufs=out_bufs) as outp,
            tc.tile_pool(name="ps", bufs=psum_bufs, space="PSUM") as psp,
        ):
            w_t = wpool.tile([128, 2, KS, KS, 2, 128], fp8)

            # PE warmup: keep the tensor engine busy through the initial DMA
            # wait so the HAM clock gate is at 8/8 when real matmuls start.
            # Writes only a scratch PSUM bank that is never read.
            if warmup:
                warm_src = wpool.tile([128, 64], fp8, name="warm_src")
                nc.vector.memset(warm_src[:], 1.0)
                warm_ps = psp.tile([128, NFREE], f32, name="warm_ps", tag="ps")
                for _ in range(warmup):
                    nc.tensor.matmul(
                        warm_ps[0:64, 0:64],
                        lhsT=warm_src[:, 0:64],
                        rhs=warm_src[:, 0:64],
                        start=True,
                        stop=True,
                    )

            # padded sign(x) in channel-pair-interleaved layout:
            # xp[k, f, i] = sign(x)[i*128 + k, spatial f]  (f in padded coords)
            pads = [
                padp.tile([128, PADF, 2], fp8, name=f"padp{ph}", tag=f"padp{ph}")
                for ph in range(pad_bufs)
            ]
            for ph in range(pad_bufs):
                xp = pads[ph]
                # zero only the padding border (the interior is rewritten by
                # Sign every image): head = row 0 + (row1,col0); the seam
                # [row r col 57 .. row r+1 col 0] for r=1..55 (4 fp8 els each);
                # tail = (row56,col57) onward through rows 57-58.
                nc.vector.memset(xp[:, 0 : PW + 1, :], 0.0)
                seam = xp.rearrange("p (a b) i -> p a b i", b=PW)
                nc.vector.memset(seam[:, 1:56, W + 1 : W + 2, :], 0.0)
                nc.vector.memset(seam[:, 1:57, 0:1, :], 0.0)
                nc.vector.memset(xp[:, 56 * PW + W + 1 :, :], 0.0)

            if fused_in:
                nc.sync.dma_start(out=w_t[:, 0], in_=w_d[0])
                nc.sync.dma_start(out=w_t[:, 1], in_=w_d[1])

            for rep in range(repeat):
                for img in range(BPC):
                    ph = img % pad_bufs
                    xp = pads[ph]
                    xp4 = xp.rearrange("p (a b) i -> p a b i", b=PW)
                    if fused_in:
                        xf = xfp.tile(
                            [128, 2, H, W], f32, name=f"xff{img}", tag="xf"
                        )
                        nc.sync.dma_start(
                            out=xf[:],
                            in_=x_d[img].rearrange("i p a b -> p i a b"),
                        )
                        for ib in range(2):
                            nc.scalar.sign(
                                xp4[:, 1 : H + 1, 1 : W + 1, ib], xf[:, ib]
                            )
                    else:
                        if img == 0 and rep == 0:
                            # tiny leading slabs: the first (split) 4-row
                            # matmul group only needs rows 0-4, so the PE
                            # starts ~4.3us in; every DMA->compute edge pays
                            # a fixed 900ns completion-semaphore delay, so
                            # small first transfers matter
                            bounds = list(first_bounds)
                        else:
                            sl = H // in_split
                            bounds = [s * sl for s in range(in_split)] + [H]
                        for s, (r0, r1) in enumerate(
                            zip(bounds[:-1], bounds[1:])
                        ):
                            # one casting SWDGE DMA per slab, both channel
                            # blocks fused: the Pool engine converts fp32 ->
                            # bf16 in flight (sign bits preserved exactly),
                            # halving the modeled transfer and the SBUF
                            # footprint.  Fusing the pair keeps descriptor
                            # generation (~1.04us) under the transfer
                            # (~1.1us for 14 rows), so the stream is dense.
                            xf = xfp.tile(
                                [128, 2, r1 - r0, W], bf16,
                                name=f"xf{img}{s}", tag="xf",
                            )
                            nc.gpsimd.dma_start(
                                out=xf[:],
                                in_=x_d[img, :, :, r0:r1].rearrange(
                                    "i p a b -> p i a b"
                                ),
                            )
                            # one fused Sign for both channel blocks (the
                            # pad layout interleaves them innermost anyway)
                            nc.scalar.sign(
                                xp4[:, 1 + r0 : 1 + r1, 1 : W + 1, :],
                                xf.rearrange("p i r w -> p r w i"),
                            )
                            if img == 0 and rep == 0 and s == 0:
                                # weights ride the (otherwise idle) SP HWDGE
                                # ring; ob=0 split per-kh so the first
                                # matmul's taps land first
                                for kh in range(KS):
                                    nc.sync.dma_start(
                                        out=w_t[:, 0, kh], in_=w_d[0][:, kh]
                                    )
                                nc.sync.dma_start(out=w_t[:, 1], in_=w_d[1])

                for img in range(BPC):
                    ph = img % pad_bufs
                    xp = pads[ph]
                    _emit_image_compute(
                        nc, mybir, psp, outp, w_t, xp, y_d, img, out_every,
                        f16, tail_ring, flush_ring,
                        ob_interleave=(img < ob_interleave_n),
                        split_c0=split_c0,
                    )
    nc.compile()
    return nc


def _emit_image_compute(
    nc,
    mybir,
    psp,
    outp,
    w_t,
    xp,
    y_d,
    img,
    out_every,
    o_dt,
    tail_ring="sync",
    flush_ring="sync",
    ob_interleave=False,
    split_c0=True,
):
    """Emit matmuls + PSUM drain + output DMA for one image.

    ob_interleave: do chunk c for both output blocks before chunk c+1 —
    halves the PE's input-row consumption rate, so early images that are
    still streaming in from HBM don't starve the PE.
    """
    f32 = mybir.dt.float32
    o_sb = [
        outp.tile([128, H, W], o_dt, name=f"osb{img}{ob}", tag="osb")
        for ob in range(2)
    ]
    done = [0, 0]

    xp4 = xp.rearrange("p (a b) i -> p a b i", b=PW)

    def emit_rows(ob, h0, h1, tag):
        """matmul group + PSUM drain for output rows [h0, h1) of block ob."""
        nrows = h1 - h0
        ps = psp.tile([128, nrows * W], f32, name=f"ps{img}{ob}{tag}", tag="ps")
        k = 0
        for kh in range(KS):
            for kw in range(KS):
                # exact output window: rhs is a 3-free-dim AP (pair, row,
                # col) — no junk pad columns are ever computed
                rhs = xp4[:, h0 + kh : h0 + kh + nrows, kw : kw + W, :].rearrange(
                    "p r w i -> p i r w"
                )
                nc.tensor.matmul(
                    ps[:],
                    lhsT=w_t[:, ob, kh, kw],
                    rhs=rhs,
                    start=(k == 0),
                    stop=(k == 8),
                    perf_mode=mybir.MatmulPerfMode.DoubleRow,
                )
                k += 1
        nc.vector.tensor_copy(
            o_sb[ob][:, h0:h1, :],
            ps.rearrange("p (r w) -> p r w", w=W),
        )

    def emit_chunk(ob, c):
        if img == 0 and c == 0 and split_c0:
            # split the very first chunk: a 4-row group needs only input
            # rows 0-4, so the PE starts right after the first tiny slab
            emit_rows(ob, 0, 4, "0a")
            emit_rows(ob, 4, CHUNK_ROWS, "0b")
        else:
            emit_rows(ob, c * CHUNK_ROWS, (c + 1) * CHUNK_ROWS, str(c))
        last = img == BPC - 1 and ob == 1
        flush = (
            (c + 1) in (4, 6, 7)
            if last  # taper the final drain: 32/16/8-row DMAs
            else ((c + 1) % out_every == 0 or c == N_CHUNKS - 1)
        )
        if flush:
            h0, h1 = done[ob] * CHUNK_ROWS, (c + 1) * CHUNK_ROWS
            # All flushes ride the SP ring, emitted AFTER every x slab: the
            # FIFO ring gives the input stream strict priority on the shared
            # DMA engines (a y flush sneaking in between x slabs delays the
            # last image's input and with it the tail of the whole kernel).
            # By the time the SP sequencer reaches a flush, its DVE copy has
            # long completed, so the semaphore wait does not head-of-line
            # block anything.
            eng = getattr(nc, tail_ring) if last else getattr(nc, flush_ring)
            eng.dma_start(
                out=y_d[img, ob, :, h0:h1],
                in_=o_sb[ob][:, h0:h1, :],
            )
            done[ob] = c + 1

    if ob_interleave:
        for c in range(N_CHUNKS):
            for ob in range(2):
                emit_chunk(ob, c)
    else:
        for ob in range(2):
            for c in range(N_CHUNKS):
                emit_chunk(ob, c)


def _decode_weights(codebook, encoded_vector):
    bw = codebook[encoded_vector].reshape(-1)[: O_CH * I_CH * KS * KS]
    bw = bw.reshape(O_CH, I_CH, KS, KS)
    # [i_blk, k(part), kh, kw, o_blk, m] : lhsT layout (contraction on partitions)
    wt = bw.transpose(1, 2, 3, 0).reshape(2, 128, KS, KS, 2, 128)
    return np.ascontiguousarray(wt).astype(ml_dtypes.bfloat16)


def _decode_weights_fp8(codebook, encoded_vector):
    bw = codebook[encoded_vector].reshape(-1)[: O_CH * I_CH * KS * KS]
    bw = bw.reshape(O_CH, I_CH, KS, KS)
    wt = bw.transpose(1, 2, 3, 0).reshape(2, 128, KS, KS, 2, 128)
    # -> [o_blk, k(part), kh, kw, i_blk(pair), m]
    w2 = wt.transpose(4, 1, 2, 3, 0, 5)
    return np.ascontiguousarray(w2).astype(ml_dtypes.float8_e4m3)


def kernel(x, weight, codebook, encoded_vector):
    global _BUILT, LAST_RESULT
    from concourse import bass_utils

    x = np.ascontiguousarray(np.asarray(x, dtype=np.float32))
    codebook = np.asarray(codebook, dtype=np.float32)
    encoded_vector = np.asarray(encoded_vector)

    use_bf16 = os.environ.get("KERNEL_VARIANT", "fp8") == "bf16"
    if _BUILT is None:
        _BUILT = _build() if use_bf16 else _build_fp8()
    nc = _BUILT

    if use_bf16:
        wt = _decode_weights(codebook, encoded_vector)
    else:
        wt = _decode_weights_fp8(codebook, encoded_vector)
    x8 = x.reshape(N_CORES, BPC, 2, 128, H, W)
    in_maps = [{"x": x8[i], "w": wt} for i in range(N_CORES)]

    trace = bool(int(os.environ.get("KERNEL_TRACE", "0")))

    def _run(tr):
        return bass_utils.run_bass_kernel_spmd(
            nc, in_maps, core_ids=list(range(N_CORES)), trace=tr
        )

    res = None
    for attempt in range(3):
        try:
            res = _run(trace)
            break
        except ModuleNotFoundError:
            # axon client without the NTFF profile hook: disable tracing
            os.environ["BASS_NEVER_TRACE"] = "1"
            trace = False
        except Exception:
            # transient device errors (NRT_EXEC_UNIT_UNRECOVERABLE) recover
            # on retry
            if attempt == 2:
                raise
            time.sleep(5)
    if res is None:
        res = _run(trace)
    LAST_RESULT = res
    y = np.stack([res.results[i]["y"] for i in range(N_CORES)], axis=0)
    return np.ascontiguousarray(y.reshape(B, O_CH, H, W).astype(np.float32))



# revision 57
# speedup vs baseline: 1.3559x; 1.0009x over previous
"""Trainium2 Bass kernel for nn_CBNNConv2d (binary 3x3 conv, 256ch, 56x56).

Math: the STE forward collapses to  y = conv2d(sign(x), bw)  where
bw = codebook[encoded_vector] reshaped to (O, I, 3, 3), entries +/-1.
The latent `weight` input cancels out of the forward value, and
(sign(x) - clip(x)) + clip(x) rounds back to exactly sign(x) in fp32 —
so the forward is an exact integer convolution of +/-1 operands.
+/-1 is exactly representable in fp8e4, and all partial sums are small
integers (|y| <= 2304), so fp32 PSUM accumulation is exact and an fp16
output loses essentially nothing (integers <= 2048 are exact in fp16;
measured rel err ~5e-10 vs the fp32 reference, identical to an fp32
output — the residual comes from the reference's own rounding of wb).

Sharding: data-parallel over batch: 32 images -> 8 cores x 4 images.
The tiny codebook decode runs on host; decoded +/-1 weights are cast to
fp8e4 and replicated to every core (0.58 MB).

Per core the shot is PE-bound: 504 DoubleRow matmuls x 448 cols x 0.5
cyc/col at 2.4 GHz = 47 us of tensor-engine time, run gap-free between
a ~5.4 us data-latency head and a ~4 us drain tail (cost-model 56.5
us/shot vs 76.4 us for the original fp32-I/O baseline):
  - x streams in row-slabs as CASTING SWDGE DMAs on the Pool ring: the
    software DGE converts fp32 -> bf16 in flight (sign bits preserved
    exactly), halving both the modeled transfer time and SBUF traffic;
    both channel blocks ride one DMA per slab so descriptor generation
    (~1.04 us serial on the Pool engine) stays under the transfer time.
    DMA totals: 6.42 MB x in (bf16) + 6.42 MB y out (fp16) + 0.58 MB
    weights = 13.4 MB =~ 37 us at the 360 B/ns serialized DMA device —
    comfortably under the PE time.
  - one fused ScalarE Sign per slab (both channel blocks in one op)
    writes fp8 +/-1 into a zero-padded channel-pair-interleaved layout
    xp[k, f, i] = sign(x)[i*128+k, f] (row pitch 58, borders zeroed
    once); 4 pad buffers = no WAR stalls between images.  Every
    DMA->compute edge pays a fixed 900 ns completion-semaphore delay,
    so the head uses a small 10-row first slab.
  - the weight DMA rides the otherwise-idle SP HWDGE ring, ob=0 half
    split per-kh so the first matmul's taps land first
  - conv as matmuls: per output-row chunk (8 rows x 56 cols, N=448), 9
    DoubleRow matmuls (one per 3x3 tap, K=256 contraction via fp8
    pairs: 2 weights/PE cell) accumulate into one PSUM bank; the rhs is
    a 3-free-dim AP (pair, row, col) over the padded buffer, so the 2
    pad columns per row are never computed; the two output-channel
    blocks of a chunk are interleaved so the PE's input-row consumption
    never outruns the arriving slabs
  - DVE copies PSUM -> SBUF fp16; output DMAs ride the SP ring (their
    semaphore waits are satisfied by dispatch time, so nothing
    head-of-line blocks), with the final flushes tapered 32/16/8 rows
    to shrink the drain tail.  8 output buffers keep compute from ever
    waiting on flushes.
"""

import os
import time

import numpy as np
import ml_dtypes

O_CH, I_CH, KS = 256, 256, 3
B, H, W = 32, 56, 56
N_CORES = 8
BPC = B // N_CORES  # images per core
PW = H + 2  # padded row pitch = 58
PAD_ROWS = 59  # 58 rows touched + 1 extra row for the +2 tap overrun
PADF = PAD_ROWS * PW  # flat padded length per channel
CHUNK_ROWS = 8
N_CHUNKS = H // CHUNK_ROWS  # 7
NFREE = CHUNK_ROWS * PW  # 464 (<= 512 fp32 per PSUM bank)
NOUT = CHUNK_ROWS * W  # 448: matmul free dim without the 2 junk cols/row

_BUILT = None
LAST_RESULT = None


def _build():
    import concourse.tile as tile
    from concourse import bacc, mybir

    f32 = mybir.dt.float32
    bf16 = mybir.dt.bfloat16

    nc = bacc.Bacc(
        "TRN2",
        target_bir_lowering=False,
        debug=False,
        num_devices=N_CORES,
    )
    x_d = nc.dram_tensor("x", [BPC, 2, 128, H, W], f32, kind="ExternalInput").ap()
    w_d = nc.dram_tensor(
        "w", [2, 128, KS, KS, 2, 128], bf16, kind="ExternalInput"
    ).ap()
    y_d = nc.dram_tensor("y", [BPC, 2, 128, H, W], f32, kind="ExternalOutput").ap()

    with tile.TileContext(nc) as tc:
        with (
            tc.tile_pool(name="wpool", bufs=1) as wpool,
            tc.tile_pool(name="xf", bufs=3) as xfp,
            tc.tile_pool(name="pads", bufs=1) as padp,
            tc.tile_pool(name="outp", bufs=3) as outp,
            tc.tile_pool(name="ps", bufs=4, space="PSUM") as psp,
        ):
            w_t = wpool.tile([128, 2, KS, KS, 2, 128], bf16)
            for ib in range(2):
                nc.sync.dma_start(out=w_t[:, ib], in_=w_d[ib])

            # persistent zero-padded sign(x) buffers: [i_blk][phase]
            pads = [
                [
                    padp.tile(
                        [128, PADF], bf16, name=f"pad{ib}{ph}", tag=f"pad{ib}{ph}"
                    )
                    for ph in range(2)
                ]
                for ib in range(2)
            ]
            for ib in range(2):
                for ph in range(2):
                    nc.vector.memset(pads[ib][ph][:], 0.0)

            for img in range(BPC):
                ph = img % 2
                for ib in range(2):
                    xf = xfp.tile([128, H, W], f32)
                    nc.sync.dma_start(out=xf[:], in_=x_d[img, ib])
                    interior = pads[ib][ph].rearrange("p (a b) -> p a b", b=PW)[
                        :, 1 : H + 1, 1 : W + 1
                    ]
                    nc.scalar.sign(interior, xf[:])
                for ob in range(2):
                    o_sb = outp.tile([128, H, W], f32)
                    for c in range(N_CHUNKS):
                        ps = psp.tile([128, NFREE], f32)
                        k = 0
                        for ib in range(2):
                            for kh in range(KS):
                                for kw in range(KS):
                                    off = c * NFREE + kh * PW + kw
                                    nc.tensor.matmul(
                                        ps[:],
                                        lhsT=w_t[:, ib, kh, kw, ob, :],
                                        rhs=pads[ib][ph][:, off : off + NFREE],
                                        start=(k == 0),
                                        stop=(k == 17),
                                    )
                                    k += 1
                        psv = ps.rearrange("p (r w) -> p r w", w=PW)
                        nc.vector.tensor_copy(
                            o_sb[:, c * CHUNK_ROWS : (c + 1) * CHUNK_ROWS, :],
                            psv[:, :, 0:W],
                        )
                    nc.sync.dma_start(out=y_d[img, ob], in_=o_sb[:])
    nc.compile()
    return nc


def _build_fp8(
    repeat=1,
    in_split=4,
    out_every=7,
    psum_bufs=8,
    xf_bufs=8,
    out_bufs=8,
    pad_bufs=4,
    warmup=100,
    tail_ring="sync",
    flush_ring="sync",
    ob_interleave_n=3,
    first_bounds=(0, 10, 18, 26, 34, 42, 56),
    split_c0=False,
):
    """fp8e4 DoubleRow variant: channels 0-127 pair with 128-255 on the same
    PE row (2 fp8 weights/cell, 2 MACs/cycle) -> K=256 contraction per matmul,
    9 matmuls per output chunk instead of 18. +/-1 is exact in fp8e4.

    in_split: split each image's input DMA+sign into row-slabs so the PE can
    start on early chunks before the whole image is staged.
    out_every: DMA the output every `out_every` chunks to shrink the drain tail.
    """
    import concourse.tile as tile
    from concourse import bacc, mybir

    f32 = mybir.dt.float32
    f16 = mybir.dt.float16
    bf16 = mybir.dt.bfloat16
    fp8 = mybir.dt.float8e4

    nc = bacc.Bacc(
        "TRN2",
        target_bir_lowering=False,
        debug=False,
        num_devices=N_CORES,
    )
    x_d = nc.dram_tensor("x", [BPC, 2, 128, H, W], f32, kind="ExternalInput").ap()
    # ob-major so each output-channel block's weights are one contiguous DMA:
    # the first matmul only needs the ob=0 half, so ob=1 streams in later.
    w_d = nc.dram_tensor(
        "w", [2, 128, KS, KS, 2, 128], fp8, kind="ExternalInput"
    ).ap()
    # fp16 output: every conv value is an integer in [-2304, 2304]; fp16 is
    # exact up to 2048 and rounds to even multiples of 2 beyond, so the
    # added error is ~0 while the output DMA traffic halves.
    y_d = nc.dram_tensor("y", [BPC, 2, 128, H, W], f16, kind="ExternalOutput").ap()

    fused_in = in_split == 0  # one 3.2MB DMA per image (both channel blocks)
    if not fused_in:
        assert H % in_split == 0

    with tile.TileContext(nc) as tc:
        with (
            tc.tile_pool(name="wpool", bufs=1) as wpool,
            tc.tile_pool(name="xf", bufs=xf_bufs) as xfp,
            tc.tile_pool(name="pads", bufs=1) as padp,
            tc.tile_pool(name="outp", bufs=out_bufs) as outp,
            tc.tile_pool(name="ps", bufs=psum_bufs, space="PSUM") as psp,
        ):
            w_t = wpool.tile([128, 2, KS, KS, 2, 128], fp8)

            # PE warmup: keep the tensor engine busy through the initial DMA
            # wait so the HAM clock gate is at 8/8 when real matmuls start.
            # Writes only a scratch PSUM bank that is never read.
            if warmup:
                warm_src = wpool.tile([128, 64], fp8, name="warm_src")
                nc.vector.memset(warm_src[:], 1.0)
                warm_ps = psp.tile([128, NFREE], f32, name="warm_ps", tag="ps")
                for _ in range(warmup):
                    nc.tensor.matmul(
                        warm_ps[0:64, 0:64],
                        lhsT=warm_src[:, 0:64],
                        rhs=warm_src[:, 0:64],
                        start=True,
                        stop=True,
                    )

            # padded sign(x) in channel-pair-interleaved layout:
            # xp[k, f, i] = sign(x)[i*128 + k, spatial f]  (f in padded coords)
            pads = [
                padp.tile([128, PADF, 2], fp8, name=f"padp{ph}", tag=f"padp{ph}")
                for ph in range(pad_bufs)
            ]
            for ph in range(pad_bufs):
                xp = pads[ph]
                # zero only the padding border (the interior is rewritten by
                # Sign every image): head = row 0 + (row1,col0); the seam
                # [row r col 57 .. row r+1 col 0] for r=1..55 (4 fp8 els each);
                # tail = (row56,col57) onward through rows 57-58.
                nc.vector.memset(xp[:, 0 : PW + 1, :], 0.0)
                seam = xp.rearrange("p (a b) i -> p a b i", b=PW)
                nc.vector.memset(seam[:, 1:56, W + 1 : W + 2, :], 0.0)
                nc.vector.memset(seam[:, 1:57, 0:1, :], 0.0)
                nc.vector.memset(xp[:, 56 * PW + W + 1 :, :], 0.0)

            if fused_in:
                nc.sync.dma_start(out=w_t[:, 0], in_=w_d[0])
                nc.sync.dma_start(out=w_t[:, 1], in_=w_d[1])

            for rep in range(repeat):
                for img in range(BPC):
                    ph = img % pad_bufs
                    xp = pads[ph]
                    xp4 = xp.rearrange("p (a b) i -> p a b i", b=PW)
                    if fused_in:
                        xf = xfp.tile(
                            [128, 2, H, W], f32, name=f"xff{img}", tag="xf"
                        )
                        nc.sync.dma_start(
                            out=xf[:],
                            in_=x_d[img].rearrange("i p a b -> p i a b"),
                        )
                        for ib in range(2):
                            nc.scalar.sign(
                                xp4[:, 1 : H + 1, 1 : W + 1, ib], xf[:, ib]
                            )
                    else:
                        if img == 0 and rep == 0:
                            # tiny leading slabs: the first (split) 4-row
                            # matmul group only needs rows 0-4, so the PE
                            # starts ~4.3us in; every DMA->compute edge pays
                            # a fixed 900ns completion-semaphore delay, so
                            # small first transfers matter
                            bounds = list(first_bounds)
                        else:
                            sl = H // in_split
                            bounds = [s * sl for s in range(in_split)] + [H]
                        for s, (r0, r1) in enumerate(
                            zip(bounds[:-1], bounds[1:])
                        ):
                            # one casting SWDGE DMA per slab, both channel
                            # blocks fused: the Pool engine converts fp32 ->
                            # bf16 in flight (sign bits preserved exactly),
                            # halving the modeled transfer and the SBUF
                            # footprint.  Fusing the pair keeps descriptor
                            # generation (~1.04us) under the transfer
                            # (~1.1us for 14 rows), so the stream is dense.
                            xf = xfp.tile(
                                [128, 2, r1 - r0, W], bf16,
                                name=f"xf{img}{s}", tag="xf",
                            )
                            nc.gpsimd.dma_start(
                                out=xf[:],
                                in_=x_d[img, :, :, r0:r1].rearrange(
                                    "i p a b -> p i a b"
                                ),
                            )
                            # one fused Sign for both channel blocks (the
                            # pad layout interleaves them innermost anyway)
                            nc.scalar.sign(
                                xp4[:, 1 + r0 : 1 + r1, 1 : W + 1, :],
                                xf.rearrange("p i r w -> p r w i"),
                            )
                            if img == 0 and rep == 0 and s == 0:
                                # weights ride the (otherwise idle) SP HWDGE
                                # ring; ob=0 split per-kh so the first
                                # matmul's taps land first
                                for kh in range(KS):
                                    nc.sync.dma_start(
                                        out=w_t[:, 0, kh], in_=w_d[0][:, kh]
                                    )
                                nc.sync.dma_start(out=w_t[:, 1], in_=w_d[1])

                for img in range(BPC):
                    ph = img % pad_bufs
                    xp = pads[ph]
                    _emit_image_compute(
                        nc, mybir, psp, outp, w_t, xp, y_d, img, out_every,
                        f16, tail_ring, flush_ring,
                        ob_interleave=(img < ob_interleave_n),
                        split_c0=split_c0,
                    )
    nc.compile()
    return nc


def _emit_image_compute(
    nc,
    mybir,
    psp,
    outp,
    w_t,
    xp,
    y_d,
    img,
    out_every,
    o_dt,
    tail_ring="sync",
    flush_ring="sync",
    ob_interleave=False,
    split_c0=True,
):
    """Emit matmuls + PSUM drain + output DMA for one image.

    ob_interleave: do chunk c for both output blocks before chunk c+1 —
    halves the PE's input-row consumption rate, so early images that are
    still streaming in from HBM don't starve the PE.
    """
    f32 = mybir.dt.float32
    o_sb = [
        outp.tile([128, H, W], o_dt, name=f"osb{img}{ob}", tag="osb")
        for ob in range(2)
    ]
    done = [0, 0]

    xp4 = xp.rearrange("p (a b) i -> p a b i", b=PW)

    def emit_rows(ob, h0, h1, tag):
        """matmul group + PSUM drain for output rows [h0, h1) of block ob."""
        nrows = h1 - h0
        ps = psp.tile([128, nrows * W], f32, name=f"ps{img}{ob}{tag}", tag="ps")
        k = 0
        for kh in range(KS):
            for kw in range(KS):
                # exact output window: rhs is a 3-free-dim AP (pair, row,
                # col) — no junk pad columns are ever computed
                rhs = xp4[:, h0 + kh : h0 + kh + nrows, kw : kw + W, :].rearrange(
                    "p r w i -> p i r w"
                )
                nc.tensor.matmul(
                    ps[:],
                    lhsT=w_t[:, ob, kh, kw],
                    rhs=rhs,
                    start=(k == 0),
                    stop=(k == 8),
                    perf_mode=mybir.MatmulPerfMode.DoubleRow,
                )
                k += 1
        nc.vector.tensor_copy(
            o_sb[ob][:, h0:h1, :],
            ps.rearrange("p (r w) -> p r w", w=W),
        )

    def emit_chunk(ob, c):
        if img == 0 and c == 0 and split_c0:
            # split the very first chunk: a 4-row group needs only input
            # rows 0-4, so the PE starts right after the first tiny slab
            emit_rows(ob, 0, 4, "0a")
            emit_rows(ob, 4, CHUNK_ROWS, "0b")
        else:
            emit_rows(ob, c * CHUNK_ROWS, (c + 1) * CHUNK_ROWS, str(c))
        last = img == BPC - 1 and ob == 1
        flush = (
            (c + 1) in (4, 6, 7)
            if last  # taper the final drain: 32/16/8-row DMAs
            else ((c + 1) % out_every == 0 or c == N_CHUNKS - 1)
        )
        if flush:
            h0, h1 = done[ob] * CHUNK_ROWS, (c + 1) * CHUNK_ROWS
            # All flushes ride the SP ring, emitted AFTER every x slab: the
            # FIFO ring gives the input stream strict priority on the shared
            # DMA engines (a y flush sneaking in between x slabs delays the
            # last image's input and with it the tail of the whole kernel).
            # By the time the SP sequencer reaches a flush, its DVE copy has
            # long completed, so the semaphore wait does not head-of-line
            # block anything.
            eng = getattr(nc, tail_ring) if last else getattr(nc, flush_ring)
            eng.dma_start(
                out=y_d[img, ob, :, h0:h1],
                in_=o_sb[ob][:, h0:h1, :],
            )
            done[ob] = c + 1

    if ob_interleave:
        for c in range(N_CHUNKS):
            for ob in range(2):
                emit_chunk(ob, c)
    else:
        for ob in range(2):
            for c in range(N_CHUNKS):
                emit_chunk(ob, c)


def _decode_weights(codebook, encoded_vector):
    bw = codebook[encoded_vector].reshape(-1)[: O_CH * I_CH * KS * KS]
    bw = bw.reshape(O_CH, I_CH, KS, KS)
    # [i_blk, k(part), kh, kw, o_blk, m] : lhsT layout (contraction on partitions)
    wt = bw.transpose(1, 2, 3, 0).reshape(2, 128, KS, KS, 2, 128)
    return np.ascontiguousarray(wt).astype(ml_dtypes.bfloat16)


def _decode_weights_fp8(codebook, encoded_vector):
    bw = codebook[encoded_vector].reshape(-1)[: O_CH * I_CH * KS * KS]
    bw = bw.reshape(O_CH, I_CH, KS, KS)
    wt = bw.transpose(1, 2, 3, 0).reshape(2, 128, KS, KS, 2, 128)
    # -> [o_blk, k(part), kh, kw, i_blk(pair), m]
    w2 = wt.transpose(4, 1, 2, 3, 0, 5)
    return np.ascontiguousarray(w2).astype(ml_dtypes.float8_e4m3)


def kernel(x, weight, codebook, encoded_vector):
    global _BUILT, LAST_RESULT
    from concourse import bass_utils

    x = np.ascontiguousarray(np.asarray(x, dtype=np.float32))
    codebook = np.asarray(codebook, dtype=np.float32)
    encoded_vector = np.asarray(encoded_vector)

    use_bf16 = os.environ.get("KERNEL_VARIANT", "fp8") == "bf16"
    if _BUILT is None:
        _BUILT = _build() if use_bf16 else _build_fp8()
    nc = _BUILT

    if use_bf16:
        wt = _decode_weights(codebook, encoded_vector)
    else:
        wt = _decode_weights_fp8(codebook, encoded_vector)
    x8 = x.reshape(N_CORES, BPC, 2, 128, H, W)
    in_maps = [{"x": x8[i], "w": wt} for i in range(N_CORES)]

    trace = bool(int(os.environ.get("KERNEL_TRACE", "0")))

    def _run(tr):
        return bass_utils.run_bass_kernel_spmd(
            nc, in_maps, core_ids=list(range(N_CORES)), trace=tr
        )

    res = None
    for attempt in range(3):
        try:
            res = _run(trace)
            break
        except ModuleNotFoundError:
            # axon client without the NTFF profile hook: disable tracing
            os.environ["BASS_NEVER_TRACE"] = "1"
            trace = False
        except Exception:
            # transient device errors (NRT_EXEC_UNIT_UNRECOVERABLE) recover
            # on retry
            if attempt == 2:
                raise
            time.sleep(5)
    if res is None:
        res = _run(trace)
    LAST_RESULT = res
    y = np.stack([res.results[i]["y"] for i in range(N_CORES)], axis=0)
    return np.ascontiguousarray(y.reshape(B, O_CH, H, W).astype(np.float32))



# revision 60
# speedup vs baseline: 1.3628x; 1.0051x over previous
"""Trainium2 Bass kernel for nn_CBNNConv2d (binary 3x3 conv, 256ch, 56x56).

Math: the STE forward collapses to  y = conv2d(sign(x), bw)  where
bw = codebook[encoded_vector] reshaped to (O, I, 3, 3), entries +/-1.
The latent `weight` input cancels out of the forward value, and
(sign(x) - clip(x)) + clip(x) rounds back to exactly sign(x) in fp32 —
so the forward is an exact integer convolution of +/-1 operands.
+/-1 is exactly representable in fp8e4, and all partial sums are small
integers (|y| <= 2304), so fp32 PSUM accumulation is exact and an fp16
output loses essentially nothing (integers <= 2048 are exact in fp16;
measured rel err ~5e-10 vs the fp32 reference, identical to an fp32
output — the residual comes from the reference's own rounding of wb).

Sharding: data-parallel over batch: 32 images -> 8 cores x 4 images.
The tiny codebook decode runs on host; decoded +/-1 weights are cast to
fp8e4 and replicated to every core (0.58 MB).

Per core the shot is PE-bound: 504 DoubleRow matmuls x 448 cols x 0.5
cyc/col at 2.4 GHz = 47 us of tensor-engine time, run gap-free between
a ~5.4 us data-latency head and a ~4 us drain tail (cost-model 56.5
us/shot vs 76.4 us for the original fp32-I/O baseline):
  - x streams in row-slabs as CASTING SWDGE DMAs on the Pool ring: the
    software DGE converts fp32 -> bf16 in flight (sign bits preserved
    exactly), halving both the modeled transfer time and SBUF traffic;
    both channel blocks ride one DMA per slab so descriptor generation
    (~1.04 us serial on the Pool engine) stays under the transfer time.
    DMA totals: 6.42 MB x in (bf16) + 6.42 MB y out (fp16) + 0.58 MB
    weights = 13.4 MB =~ 37 us at the 360 B/ns serialized DMA device —
    comfortably under the PE time.
  - one fused ScalarE Sign per slab (both channel blocks in one op)
    writes fp8 +/-1 into a zero-padded channel-pair-interleaved layout
    xp[k, f, i] = sign(x)[i*128+k, f] (row pitch 58, borders zeroed
    once); 4 pad buffers = no WAR stalls between images.  Every
    DMA->compute edge pays a fixed 900 ns completion-semaphore delay,
    so the head uses a small 10-row first slab.
  - the weight DMA rides the otherwise-idle SP HWDGE ring, ob=0 half
    split per-kh so the first matmul's taps land first
  - conv as matmuls: per output-row chunk (8 rows x 56 cols, N=448), 9
    DoubleRow matmuls (one per 3x3 tap, K=256 contraction via fp8
    pairs: 2 weights/PE cell) accumulate into one PSUM bank; the rhs is
    a 3-free-dim AP (pair, row, col) over the padded buffer, so the 2
    pad columns per row are never computed; the two output-channel
    blocks of a chunk are interleaved so the PE's input-row consumption
    never outruns the arriving slabs
  - DVE copies PSUM -> SBUF fp16; output DMAs ride the SP ring (their
    semaphore waits are satisfied by dispatch time, so nothing
    head-of-line blocks), with the final flushes tapered 32/16/8 rows
    to shrink the drain tail.  8 output buffers keep compute from ever
    waiting on flushes.
"""

import os
import time

import numpy as np
import ml_dtypes

O_CH, I_CH, KS = 256, 256, 3
B, H, W = 32, 56, 56
N_CORES = 8
BPC = B // N_CORES  # images per core
PW = H + 2  # padded row pitch = 58
PAD_ROWS = 59  # 58 rows touched + 1 extra row for the +2 tap overrun
PADF = PAD_ROWS * PW  # flat padded length per channel
CHUNK_ROWS = 8
N_CHUNKS = H // CHUNK_ROWS  # 7
NFREE = CHUNK_ROWS * PW  # 464 (<= 512 fp32 per PSUM bank)
NOUT = CHUNK_ROWS * W  # 448: matmul free dim without the 2 junk cols/row

_BUILT = None
LAST_RESULT = None


def _build():
    import concourse.tile as tile
    from concourse import bacc, mybir

    f32 = mybir.dt.float32
    bf16 = mybir.dt.bfloat16

    nc = bacc.Bacc(
        "TRN2",
        target_bir_lowering=False,
        debug=False,
        num_devices=N_CORES,
    )
    x_d = nc.dram_tensor("x", [BPC, 2, 128, H, W], f32, kind="ExternalInput").ap()
    w_d = nc.dram_tensor(
        "w", [2, 128, KS, KS, 2, 128], bf16, kind="ExternalInput"
    ).ap()
    y_d = nc.dram_tensor("y", [BPC, 2, 128, H, W], f32, kind="ExternalOutput").ap()

    with tile.TileContext(nc) as tc:
        with (
            tc.tile_pool(name="wpool", bufs=1) as wpool,
            tc.tile_pool(name="xf", bufs=3) as xfp,
            tc.tile_pool(name="pads", bufs=1) as padp,
            tc.tile_pool(name="outp", bufs=3) as outp,
            tc.tile_pool(name="ps", bufs=4, space="PSUM") as psp,
        ):
            w_t = wpool.tile([128, 2, KS, KS, 2, 128], bf16)
            for ib in range(2):
                nc.sync.dma_start(out=w_t[:, ib], in_=w_d[ib])

            # persistent zero-padded sign(x) buffers: [i_blk][phase]
            pads = [
                [
                    padp.tile(
                        [128, PADF], bf16, name=f"pad{ib}{ph}", tag=f"pad{ib}{ph}"
                    )
                    for ph in range(2)
                ]
                for ib in range(2)
            ]
            for ib in range(2):
                for ph in range(2):
                    nc.vector.memset(pads[ib][ph][:], 0.0)

            for img in range(BPC):
                ph = img % 2
                for ib in range(2):
                    xf = xfp.tile([128, H, W], f32)
                    nc.sync.dma_start(out=xf[:], in_=x_d[img, ib])
                    interior = pads[ib][ph].rearrange("p (a b) -> p a b", b=PW)[
                        :, 1 : H + 1, 1 : W + 1
                    ]
                    nc.scalar.sign(interior, xf[:])
                for ob in range(2):
                    o_sb = outp.tile([128, H, W], f32)
                    for c in range(N_CHUNKS):
                        ps = psp.tile([128, NFREE], f32)
                        k = 0
                        for ib in range(2):
                            for kh in range(KS):
                                for kw in range(KS):
                                    off = c * NFREE + kh * PW + kw
                                    nc.tensor.matmul(
                                        ps[:],
                                        lhsT=w_t[:, ib, kh, kw, ob, :],
                                        rhs=pads[ib][ph][:, off : off + NFREE],
                                        start=(k == 0),
                                        stop=(k == 17),
                                    )
                                    k += 1
                        psv = ps.rearrange("p (r w) -> p r w", w=PW)
                        nc.vector.tensor_copy(
                            o_sb[:, c * CHUNK_ROWS : (c + 1) * CHUNK_ROWS, :],
                            psv[:, :, 0:W],
                        )
                    nc.sync.dma_start(out=y_d[img, ob], in_=o_sb[:])
    nc.compile()
    return nc


def _build_fp8(
    repeat=1,
    in_split=4,
    out_every=7,
    psum_bufs=8,
    xf_bufs=8,
    out_bufs=8,
    pad_bufs=4,
    warmup=100,
    tail_ring="sync",
    flush_ring="sync",
    ob_interleave_n=3,
    first_bounds=(0, 10, 18, 26, 34, 42, 56),
    split_c0=True,
):
    """fp8e4 DoubleRow variant: channels 0-127 pair with 128-255 on the same
    PE row (2 fp8 weights/cell, 2 MACs/cycle) -> K=256 contraction per matmul,
    9 matmuls per output chunk instead of 18. +/-1 is exact in fp8e4.

    in_split: split each image's input DMA+sign into row-slabs so the PE can
    start on early chunks before the whole image is staged.
    out_every: DMA the output every `out_every` chunks to shrink the drain tail.
    """
    import concourse.tile as tile
    from concourse import bacc, mybir

    f32 = mybir.dt.float32
    f16 = mybir.dt.float16
    bf16 = mybir.dt.bfloat16
    fp8 = mybir.dt.float8e4

    nc = bacc.Bacc(
        "TRN2",
        target_bir_lowering=False,
        debug=False,
        num_devices=N_CORES,
    )
    x_d = nc.dram_tensor("x", [BPC, 2, 128, H, W], f32, kind="ExternalInput").ap()
    # ob-major so each output-channel block's weights are one contiguous DMA:
    # the first matmul only needs the ob=0 half, so ob=1 streams in later.
    w_d = nc.dram_tensor(
        "w", [2, 128, KS, KS, 2, 128], fp8, kind="ExternalInput"
    ).ap()
    # fp16 output: every conv value is an integer in [-2304, 2304]; fp16 is
    # exact up to 2048 and rounds to even multiples of 2 beyond, so the
    # added error is ~0 while the output DMA traffic halves.
    y_d = nc.dram_tensor("y", [BPC, 2, 128, H, W], f16, kind="ExternalOutput").ap()

    fused_in = in_split == 0  # one 3.2MB DMA per image (both channel blocks)
    if not fused_in:
        assert H % in_split == 0

    with tile.TileContext(nc) as tc:
        with (
            tc.tile_pool(name="wpool", bufs=1) as wpool,
            tc.tile_pool(name="xf", bufs=xf_bufs) as xfp,
            tc.tile_pool(name="pads", bufs=1) as padp,
            tc.tile_pool(name="outp", bufs=out_bufs) as outp,
            tc.tile_pool(name="ps", bufs=psum_bufs, space="PSUM") as psp,
        ):
            w_t = wpool.tile([128, 2, KS, KS, 2, 128], fp8)

            # PE warmup: keep the tensor engine busy through the initial DMA
            # wait so the HAM clock gate is at 8/8 when real matmuls start.
            # Writes only a scratch PSUM bank that is never read.
            if warmup:
                warm_src = wpool.tile([128, 64], fp8, name="warm_src")
                nc.vector.memset(warm_src[:], 1.0)
                warm_ps = psp.tile([128, NFREE], f32, name="warm_ps", tag="ps")
                for _ in range(warmup):
                    nc.tensor.matmul(
                        warm_ps[0:64, 0:64],
                        lhsT=warm_src[:, 0:64],
                        rhs=warm_src[:, 0:64],
                        start=True,
                        stop=True,
                    )

            # padded sign(x) in channel-pair-interleaved layout:
            # xp[k, f, i] = sign(x)[i*128 + k, spatial f]  (f in padded coords)
            pads = [
                padp.tile([128, PADF, 2], fp8, name=f"padp{ph}", tag=f"padp{ph}")
                for ph in range(pad_bufs)
            ]
            for ph in range(pad_bufs):
                xp = pads[ph]
                # zero only the padding border (the interior is rewritten by
                # Sign every image): head = row 0 + (row1,col0); the seam
                # [row r col 57 .. row r+1 col 0] for r=1..55 (4 fp8 els each);
                # tail = (row56,col57) onward through rows 57-58.
                nc.vector.memset(xp[:, 0 : PW + 1, :], 0.0)
                seam = xp.rearrange("p (a b) i -> p a b i", b=PW)
                nc.vector.memset(seam[:, 1:56, W + 1 : W + 2, :], 0.0)
                nc.vector.memset(seam[:, 1:57, 0:1, :], 0.0)
                nc.vector.memset(xp[:, 56 * PW + W + 1 :, :], 0.0)

            if fused_in:
                nc.sync.dma_start(out=w_t[:, 0], in_=w_d[0])
                nc.sync.dma_start(out=w_t[:, 1], in_=w_d[1])

            for rep in range(repeat):
                for img in range(BPC):
                    ph = img % pad_bufs
                    xp = pads[ph]
                    xp4 = xp.rearrange("p (a b) i -> p a b i", b=PW)
                    if fused_in:
                        xf = xfp.tile(
                            [128, 2, H, W], f32, name=f"xff{img}", tag="xf"
                        )
                        nc.sync.dma_start(
                            out=xf[:],
                            in_=x_d[img].rearrange("i p a b -> p i a b"),
                        )
                        for ib in range(2):
                            nc.scalar.sign(
                                xp4[:, 1 : H + 1, 1 : W + 1, ib], xf[:, ib]
                            )
                    else:
                        if img == 0 and rep == 0:
                            # tiny leading slabs: the first (split) 4-row
                            # matmul group only needs rows 0-4, so the PE
                            # starts ~4.3us in; every DMA->compute edge pays
                            # a fixed 900ns completion-semaphore delay, so
                            # small first transfers matter
                            bounds = list(first_bounds)
                        else:
                            sl = H // in_split
                            bounds = [s * sl for s in range(in_split)] + [H]
                        for s, (r0, r1) in enumerate(
                            zip(bounds[:-1], bounds[1:])
                        ):
                            # one casting SWDGE DMA per slab, both channel
                            # blocks fused: the Pool engine converts fp32 ->
                            # bf16 in flight (sign bits preserved exactly),
                            # halving the modeled transfer and the SBUF
                            # footprint.  Fusing the pair keeps descriptor
                            # generation (~1.04us) under the transfer
                            # (~1.1us for 14 rows), so the stream is dense.
                            xf = xfp.tile(
                                [128, 2, r1 - r0, W], bf16,
                                name=f"xf{img}{s}", tag="xf",
                            )
                            nc.gpsimd.dma_start(
                                out=xf[:],
                                in_=x_d[img, :, :, r0:r1].rearrange(
                                    "i p a b -> p i a b"
                                ),
                            )
                            # one fused Sign for both channel blocks (the
                            # pad layout interleaves them innermost anyway).
                            # The very first slab signs in two pieces so the
                            # (split) 4-row first matmul group — which only
                            # needs input rows 0-5 — starts ~0.5us earlier.
                            if img == 0 and rep == 0 and s == 0 and split_c0:
                                sign_bounds = [r0, r0 + 6, r1]
                            else:
                                sign_bounds = [r0, r1]
                            for a0, a1 in zip(sign_bounds[:-1], sign_bounds[1:]):
                                nc.scalar.sign(
                                    xp4[:, 1 + a0 : 1 + a1, 1 : W + 1, :],
                                    xf[:, :, a0 - r0 : a1 - r0].rearrange(
                                        "p i r w -> p r w i"
                                    ),
                                )
                            if img == 0 and rep == 0 and s == 0:
                                # weights ride the (otherwise idle) SP HWDGE
                                # ring; ob=0 split per-kh so the first
                                # matmul's taps land first
                                for kh in range(KS):
                                    nc.sync.dma_start(
                                        out=w_t[:, 0, kh], in_=w_d[0][:, kh]
                                    )
                                nc.sync.dma_start(out=w_t[:, 1], in_=w_d[1])

                for img in range(BPC):
                    ph = img % pad_bufs
                    xp = pads[ph]
                    _emit_image_compute(
                        nc, mybir, psp, outp, w_t, xp, y_d, img, out_every,
                        f16, tail_ring, flush_ring,
                        ob_interleave=(img < ob_interleave_n),
                        split_c0=split_c0,
                    )
    nc.compile()
    return nc


def _emit_image_compute(
    nc,
    mybir,
    psp,
    outp,
    w_t,
    xp,
    y_d,
    img,
    out_every,
    o_dt,
    tail_ring="sync",
    flush_ring="sync",
    ob_interleave=False,
    split_c0=True,
):
    """Emit matmuls + PSUM drain + output DMA for one image.

    ob_interleave: do chunk c for both output blocks before chunk c+1 —
    halves the PE's input-row consumption rate, so early images that are
    still streaming in from HBM don't starve the PE.
    """
    f32 = mybir.dt.float32
    o_sb = [
        outp.tile([128, H, W], o_dt, name=f"osb{img}{ob}", tag="osb")
        for ob in range(2)
    ]
    done = [0, 0]

    xp4 = xp.rearrange("p (a b) i -> p a b i", b=PW)

    def emit_rows(ob, h0, h1, tag):
        """matmul group + PSUM drain for output rows [h0, h1) of block ob."""
        nrows = h1 - h0
        ps = psp.tile([128, nrows * W], f32, name=f"ps{img}{ob}{tag}", tag="ps")
        k = 0
        for kh in range(KS):
            for kw in range(KS):
                # exact output window: rhs is a 3-free-dim AP (pair, row,
                # col) — no junk pad columns are ever computed
                rhs = xp4[:, h0 + kh : h0 + kh + nrows, kw : kw + W, :].rearrange(
                    "p r w i -> p i r w"
                )
                nc.tensor.matmul(
                    ps[:],
                    lhsT=w_t[:, ob, kh, kw],
                    rhs=rhs,
                    start=(k == 0),
                    stop=(k == 8),
                    perf_mode=mybir.MatmulPerfMode.DoubleRow,
                )
                k += 1
        nc.vector.tensor_copy(
            o_sb[ob][:, h0:h1, :],
            ps.rearrange("p (r w) -> p r w", w=W),
        )

    def emit_chunk(ob, c):
        if img == 0 and c == 0 and split_c0:
            # split the very first chunk: a 4-row group needs only input
            # rows 0-4 (the first piece of the split sign), so the PE
            # starts as early as possible
            emit_rows(ob, 0, 4, "0a")
            emit_rows(ob, 4, CHUNK_ROWS, "0b")
        elif img == BPC - 1 and ob == 1 and c == N_CHUNKS - 1:
            # split the very last chunk into two PSUM groups (one flush):
            # rows 48-52's PSUM drain overlaps the final 4-row group's
            # matmuls, trimming the kernel tail
            emit_rows(ob, 48, 52, "6a")
            emit_rows(ob, 52, 56, "6b")
        else:
            emit_rows(ob, c * CHUNK_ROWS, (c + 1) * CHUNK_ROWS, str(c))
        last = img == BPC - 1 and ob == 1
        flush = (
            (c + 1) in (4, 6, 7)
            if last  # taper the final drain: 32/16/8-row DMAs
            else ((c + 1) % out_every == 0 or c == N_CHUNKS - 1)
        )
        if flush:
            h0, h1 = done[ob] * CHUNK_ROWS, (c + 1) * CHUNK_ROWS
            # All flushes ride the SP ring, emitted AFTER every x slab: the
            # FIFO ring gives the input stream strict priority on the shared
            # DMA engines (a y flush sneaking in between x slabs delays the
            # last image's input and with it the tail of the whole kernel).
            # By the time the SP sequencer reaches a flush, its DVE copy has
            # long completed, so the semaphore wait does not head-of-line
            # block anything.
            eng = getattr(nc, tail_ring) if last else getattr(nc, flush_ring)
            eng.dma_start(
                out=y_d[img, ob, :, h0:h1],
                in_=o_sb[ob][:, h0:h1, :],
            )
            done[ob] = c + 1

    if ob_interleave:
        for c in range(N_CHUNKS):
            for ob in range(2):
                emit_chunk(ob, c)
    else:
        for ob in range(2):
            for c in range(N_CHUNKS):
                emit_chunk(ob, c)


def _decode_weights(codebook, encoded_vector):
    bw = codebook[encoded_vector].reshape(-1)[: O_CH * I_CH * KS * KS]
    bw = bw.reshape(O_CH, I_CH, KS, KS)
    # [i_blk, k(part), kh, kw, o_blk, m] : lhsT layout (contraction on partitions)
    wt = bw.transpose(1, 2, 3, 0).reshape(2, 128, KS, KS, 2, 128)
    return np.ascontiguousarray(wt).astype(ml_dtypes.bfloat16)


def _decode_weights_fp8(codebook, encoded_vector):
    bw = codebook[encoded_vector].reshape(-1)[: O_CH * I_CH * KS * KS]
    bw = bw.reshape(O_CH, I_CH, KS, KS)
    wt = bw.transpose(1, 2, 3, 0).reshape(2, 128, KS, KS, 2, 128)
    # -> [o_blk, k(part), kh, kw, i_blk(pair), m]
    w2 = wt.transpose(4, 1, 2, 3, 0, 5)
    return np.ascontiguousarray(w2).astype(ml_dtypes.float8_e4m3)


def kernel(x, weight, codebook, encoded_vector):
    global _BUILT, LAST_RESULT
    from concourse import bass_utils

    x = np.ascontiguousarray(np.asarray(x, dtype=np.float32))
    codebook = np.asarray(codebook, dtype=np.float32)
    encoded_vector = np.asarray(encoded_vector)

    use_bf16 = os.environ.get("KERNEL_VARIANT", "fp8") == "bf16"
    if _BUILT is None:
        _BUILT = _build() if use_bf16 else _build_fp8()
    nc = _BUILT

    if use_bf16:
        wt = _decode_weights(codebook, encoded_vector)
    else:
        wt = _decode_weights_fp8(codebook, encoded_vector)
    x8 = x.reshape(N_CORES, BPC, 2, 128, H, W)
    in_maps = [{"x": x8[i], "w": wt} for i in range(N_CORES)]

    trace = bool(int(os.environ.get("KERNEL_TRACE", "0")))

    def _run(tr):
        return bass_utils.run_bass_kernel_spmd(
            nc, in_maps, core_ids=list(range(N_CORES)), trace=tr
        )

    res = None
    for attempt in range(3):
        try:
            res = _run(trace)
            break
        except ModuleNotFoundError:
            # axon client without the NTFF profile hook: disable tracing
            os.environ["BASS_NEVER_TRACE"] = "1"
            trace = False
        except Exception:
            # transient device errors (NRT_EXEC_UNIT_UNRECOVERABLE) recover
            # on retry
            if attempt == 2:
                raise
            time.sleep(5)
    if res is None:
        res = _run(trace)
    LAST_RESULT = res
    y = np.stack([res.results[i]["y"] for i in range(N_CORES)], axis=0)
    return np.ascontiguousarray(y.reshape(B, O_CH, H, W).astype(np.float32))

